# revision 12
# baseline (speedup 1.0000x reference)
"""BiLSTM-CRF Trainium2 kernel (flip-layout v2).

Full-input contract: kernel(**inputs) takes the unsharded numpy inputs and
returns the full [64, 512, 32, 32] float32 output. Batch (64) is sharded
across 8 NeuronCores (8 sentences per core); the embedding lookup runs on
the host (so only the gathered activations cross the wire, not the 15MB
table replicated 8x), the BiLSTM scan + emission + CRF expand run on
device, and per-core outputs are reassembled zero-copy.

Device pipeline per core (instruction-count-minimal layout):
  - x upload pre-gathered as bf16 [L*8, 128] (t-major); PE-transposed into
    xT [128 = E, L*8].
  - P windows (16 steps = 128 tokens): P[tok, gate] = x @ Wih.T + b via 2
    matmuls per 512-col half, PSUM -> SBUF fp32. Gate col order (i,f,o,g).
    The bwd direction reuses the fwd xT: window w reads fwd window
    NWIN-1-w and steps index its partition rows mirrored.
  - Scan step (both dirs): gates[8b, 1024] = h_prev.T @ Whh (4 matmuls per
    dir: 2 k-chunks x 2 psum-bank halves), + P row-slice via one DVE add
    per dir into gates_sb [8, 2048] = (dir, gate, 256). One sigmoid ACT
    covers i,f,o of both dirs (strided AP); one tanh ACT for g. LSTM cell
    update in 3 DVE ops on (gate, dir, x) views; h [8, 512] bf16 is
    PE-transposed (4x [8,128]) into h_histT [128, (t, dc4, b8)] with the
    bwd half written at slot L-1-s.
  - Emission: per 64-t block, 4 accumulating matmuls (k-chunks = dir x
    h-chunk) -> emisT [33, L*8] (row 32 == 1 via ACT bias trick).
  - CRF expand: per 128-token tile, 2 matmuls lhsT=emisT-slice vs
    Jsel_aug halves (row 32 carries transition + b_lin), 2 copies, 1 DMA.
"""

import numpy as np

VOCAB, EMB, HID, OUT = 30000, 128, 256, 32
B, L = 64, 512
NCORES = 8
BC = B // NCORES  # sentences per core = 8


# ---------------------------------------------------------------------------
# host-side prep

_WCACHE = {}


def _fingerprint(a):
    a = np.ascontiguousarray(a)
    v = a.view(np.uint8)
    return (a.shape, a.dtype.str, bytes(v.reshape(-1)[:: max(1, v.size // 64)][:64]))


def _prep_weights(inputs):
    """Pack weights (bf16) + Jsel/bias constants. Cached on input identity."""
    import ml_dtypes

    key = tuple(inputs[k].__array_interface__["data"][0]
                for k in ("Wih_f", "Whh_f", "W_lin", "transition", "embedding"))
    fp = (_fingerprint(inputs["W_lin"]), _fingerprint(inputs["Whh_f"]))
    ent = _WCACHE.get(key)
    if ent is not None and ent["fp"] == fp:
        return ent

    bf16 = ml_dtypes.bfloat16
    # torch gate order i,f,g,o -> i,f,o,g
    perm = np.concatenate([np.arange(0, 512), np.arange(768, 1024),
                           np.arange(512, 768)])

    def packdir(Wih, Whh, bih, bhh):
        Wih = np.asarray(Wih, np.float32)[perm]          # [1024, 128]
        Whh = np.asarray(Whh, np.float32)[perm]          # [1024, 256]
        b = (np.asarray(bih, np.float32) + np.asarray(bhh, np.float32))[perm]
        return (np.ascontiguousarray(Wih.T),             # [128, 1024]
                np.ascontiguousarray(Whh[:, :128].T),    # [128, 1024]
                np.ascontiguousarray(Whh[:, 128:].T),    # [128, 1024]
                b)

    wf = packdir(inputs["Wih_f"], inputs["Whh_f"], inputs["bih_f"],
                 inputs["bhh_f"])
    wb = packdir(inputs["Wih_b"], inputs["Whh_b"], inputs["bih_b"],
                 inputs["bhh_b"])

    W_lin = np.asarray(inputs["W_lin"], np.float32)       # [32, 512]
    b_lin = np.asarray(inputs["b_lin"], np.float32)
    trans = np.asarray(inputs["transition"], np.float32)

    WlinT = np.ascontiguousarray(W_lin.T)                 # [512, 32]
    Wlin_aug = np.zeros([4, 128, 33], np.float32)
    for kt in range(4):
        Wlin_aug[kt, :, :32] = WlinT[kt * 128:(kt + 1) * 128]

    # cw: [128, 6*1024 + 4*33] bf16
    cw = np.concatenate(
        [wf[0], wb[0], wf[1], wf[2], wb[1], wb[2],
         Wlin_aug.transpose(1, 0, 2).reshape(128, 132)], axis=1)
    cw = np.ascontiguousarray(cw).astype(bf16)

    cb = np.concatenate([wf[3], wb[3]])[None, :]          # [1, 2048]
    cb = np.ascontiguousarray(cb).astype(bf16)

    Jsel_aug = np.zeros([33, 1024], np.float32)
    for j in range(32):
        Jsel_aug[j, np.arange(32) * 32 + j] = 1.0
    Jsel_aug[32] = (trans + b_lin[None, :]).reshape(-1)
    emis_bias = np.zeros([33, 1], np.float32)
    emis_bias[32, 0] = 1.0
    cj = np.ascontiguousarray(np.concatenate([Jsel_aug, emis_bias], axis=1))

    emb_bf = np.asarray(inputs["embedding"], np.float32).astype(bf16)

    ent = {"cw": cw, "cb": cb, "cj": cj, "emb_bf": emb_bf, "fp": fp}
    _WCACHE[key] = ent
    return ent


def _host_prep(inputs, L_eff=L):
    ent = _prep_weights(inputs)
    sents = np.asarray(inputs["sents_tensor"])
    if sents.dtype != np.int64 and sents.dtype != np.int32:
        sents = sents.astype(np.int64)
    emb_bf = ent["emb_bf"]
    in_maps = []
    for c in range(NCORES):
        idx = sents[c * BC:(c + 1) * BC, :L_eff].T.reshape(-1)  # (t, b)
        x = np.ascontiguousarray(emb_bf[idx])          # [L*8, 128] bf16
        in_maps.append({"x": x, "cw": ent["cw"], "cb": ent["cb"],
                        "cj": ent["cj"]})
    return in_maps


# ---------------------------------------------------------------------------
# device program

def build_nc(L_eff=L, reps=1, timing=False, has_bias=True):
    import concourse.bass as bass  # noqa: F401
    import concourse.mybir as mybir
    import concourse.tile as tile
    from concourse.bacc import Bacc
    from concourse.masks import make_identity

    dt = mybir.dt
    AF = mybir.ActivationFunctionType
    OP = mybir.AluOpType

    NTOK = L_eff * BC
    WIN = 8 if L_eff >= 8 else L_eff      # steps per P window
    NWIN = L_eff // WIN
    NTW = WIN * 8                         # tokens per window
    NT = NTOK // 128                      # x tiles

    nc = Bacc()

    d_x = nc.declare_dram_parameter("x", [NTOK, 128], dt.bfloat16, False)
    d_cw = nc.declare_dram_parameter("cw", [128, 6 * 1024 + 132],
                                     dt.bfloat16, False)
    d_cb = nc.declare_dram_parameter("cb", [1, 2048], dt.bfloat16, False)
    d_cj = nc.declare_dram_parameter("cj", [33, 1025], dt.float32r, False)
    if timing:
        d_out = nc.dram_tensor("outt", [BC, L_eff, 1024], dt.float32)
        d_out_ext = nc.declare_dram_parameter("out", [1, 16], dt.float32,
                                              isOutput=True)
    else:
        d_out = nc.declare_dram_parameter("out", [BC, L_eff, 1024],
                                          dt.float32, isOutput=True)
        d_out_ext = None
    d_pb = [nc.dram_tensor(f"pb{d}", [WIN, 8, 1024], dt.bfloat16)
            for d in range(2)]

    with tile.TileContext(nc) as tc:
        with (
            tc.tile_pool(name="const", bufs=1) as const,
            tc.tile_pool(name="state", bufs=1) as state,
        ):
            ident = const.tile([128, 128], dt.float32)
            make_identity(nc, ident[:])
            ident_b = const.tile([128, 128], dt.bfloat16)
            nc.vector.tensor_copy(out=ident_b[:], in_=ident[:])
            ones1 = const.tile([1, 128], dt.bfloat16)
            nc.vector.memset(ones1[:], 1.0)

            cw_sb = const.tile([128, 6 * 1024 + 132], dt.bfloat16)
            nc.sync.dma_start(out=cw_sb[:], in_=d_cw[:])
            cb_sb = const.tile([1, 2048], dt.bfloat16)
            nc.sync.dma_start(out=cb_sb[:], in_=d_cb[:])
            cj_sb = const.tile([33, 1025], dt.float32r)
            nc.sync.dma_start(out=cj_sb[:], in_=d_cj[:])
            jsel = cj_sb[:, 0:1024]
            ebias = cj_sb[:, 1024:1025].bitcast(dt.float32)

            def wih(d):
                return cw_sb[:, d * 1024:(d + 1) * 1024]

            def whh(d, k):
                off = 2048 + (d * 2 + k) * 1024
                return cw_sb[:, off:off + 1024]

            def wlin(kt):
                off = 6144 + kt * 33
                return cw_sb[:, off:off + 33]

            xT = state.tile([128, NTOK], dt.bfloat16)
            h_histT = state.tile([128, L_eff * 32], dt.bfloat16)
            emisT = state.tile([33, NTOK], dt.float32r)

            with (
                tc.tile_pool(name="gat", bufs=1) as gat,
                tc.tile_pool(name="tp", bufs=2, space="PSUM") as tp_p,
                tc.tile_pool(name="pw", bufs=1, space="PSUM") as pw_ps_p,
                tc.tile_pool(name="gps", bufs=1, space="PSUM") as g_ps_p,
                tc.tile_pool(name="cps", bufs=2, space="PSUM") as crf_ps_p,
                tc.tile_pool(name="psb", bufs=1) as p_sb_p,
                tc.tile_pool(name="pfl", bufs=1) as p_fl_p,
                tc.tile_pool(name="sg", bufs=1) as sg_p,
                tc.tile_pool(name="av", bufs=1) as av_p,
                tc.tile_pool(name="tt", bufs=1) as tt_p,
                tc.tile_pool(name="tgc", bufs=2) as tgc_p,
                tc.tile_pool(name="tc2", bufs=1) as tc_p,
                tc.tile_pool(name="hsb", bufs=2) as h_sb_p,
                tc.tile_pool(name="csb", bufs=2) as crf_sb_p,
            ):
                # ---- x load + transpose into xT ----
                if timing:
                    nc.vector.memset(xT[:], 0.0)
                ngrp = 0 if timing else (NT + 7) // 8
                for g in range(ngrp):
                    a0 = g * 8
                    na = min(8, NT - a0)
                    gt = gat.tile([128, 128 * na], dt.bfloat16, tag="g")
                    src = d_x[:].rearrange("(a p) e -> p a e", p=128)
                    nc.sync.dma_start(
                        out=gt[:].rearrange("p (a e) -> p a e", e=128),
                        in_=src[:, a0:a0 + na, :])
                    for a in range(na):
                        pt = tp_p.tile([128, 128], dt.bfloat16, tag="t")
                        nc.tensor.transpose(
                            out=pt[:], in_=gt[:, a * 128:(a + 1) * 128],
                            identity=ident_b[:])
                        col = (a0 + a) * 128
                        if a % 2 == 0:
                            nc.vector.tensor_copy(
                                out=xT[:, col:col + 128], in_=pt[:])
                        else:
                            nc.scalar.copy(
                                out=xT[:, col:col + 128], in_=pt[:])

                P_flat = [None, None]

                def jit_window(d, w):
                    """P window -> P_flat[d] [8 b, WIN*1024] bf16.

                    Engine reads need 32-aligned partition bases, so the
                    per-step P row-slices of the [128 tok, 1024] PSUM tile
                    are shuffled to batch-partition layout via a DRAM
                    bounce (DMA moves across partitions freely).
                    """
                    win = w if d == 0 else NWIN - 1 - w
                    ps = pw_ps_p.tile([NTW, 1024], dt.float32, tag="P")
                    for hf in range(2):
                        o = hf * 512
                        if has_bias:
                            nc.tensor.matmul(
                                out=ps[:, o:o + 512], lhsT=ones1[:, 0:NTW],
                                rhs=cb_sb[0:1,
                                          d * 1024 + o:d * 1024 + o + 512],
                                start=True, stop=False)
                        nc.tensor.matmul(
                            out=ps[:, o:o + 512],
                            lhsT=xT[:, win * NTW:(win + 1) * NTW],
                            rhs=wih(d)[:, o:o + 512],
                            start=not has_bias, stop=True)
                    sb = p_sb_p.tile([NTW, 1024], dt.bfloat16, tag=f"p{d}")
                    if d == 0:
                        nc.scalar.copy(out=sb[:], in_=ps[:])
                    else:
                        nc.vector.tensor_copy(out=sb[:], in_=ps[:])
                    nc.sync.dma_start(
                        out=d_pb[d][:].rearrange("t b g -> (t b) g"),
                        in_=sb[:])
                    fl = p_fl_p.tile([8, WIN * 1024], dt.bfloat16,
                                     tag=f"f{d}")
                    nc.sync.dma_start(
                        out=fl[:].rearrange("p (t g) -> p t g", g=1024),
                        in_=d_pb[d][:].rearrange("t b g -> b t g"))
                    P_flat[d] = fl

                tgc_prev = [None]

                def scan_step(s):
                    t_l = s % WIN
                    if t_l == 0:
                        jit_window(0, s // WIN)
                        jit_window(1, s // WIN)
                    colf = slice(t_l * 1024, t_l * 1024 + 1024)
                    colb = slice((WIN - 1 - t_l) * 1024,
                                 (WIN - t_l) * 1024)
                    sg = sg_p.tile([8, 2048], dt.float32, tag="s")
                    for d in range(2):
                        col = colf if d == 0 else colb
                        if s == 0:
                            if d == 0:
                                nc.scalar.copy(out=sg[:, 0:1024],
                                               in_=P_flat[0][:, col])
                            else:
                                nc.vector.tensor_copy(out=sg[:, 1024:2048],
                                                      in_=P_flat[1][:, col])
                            continue
                        slot = (s - 1) if d == 0 else (L_eff - s)
                        g_ps = g_ps_p.tile([8, 1024], dt.float32, tag="g")
                        for hf in range(2):
                            o = hf * 512
                            for k in range(2):
                                lh = h_histT[:, slot * 32 + (d * 2 + k) * 8:
                                             slot * 32 + (d * 2 + k) * 8 + 8]
                                nc.tensor.matmul(
                                    out=g_ps[:, o:o + 512], lhsT=lh,
                                    rhs=whh(d, k)[:, o:o + 512],
                                    start=(k == 0), stop=(k == 1))
                        nc.vector.tensor_tensor(
                            out=sg[:, d * 1024:(d + 1) * 1024], in0=g_ps[:],
                            in1=P_flat[d][:, col], op=OP.add)
                    # nonlinearities; sg layout (d2, g4, x256), g = i,f,o,g
                    av = av_p.tile([8, 2048], dt.float32, tag="a")
                    sgv = sg[:].rearrange("p (d g x) -> p d g x", d=2, g=4)
                    avv = av[:].rearrange("p (d g x) -> p d g x", d=2, g=4)
                    nc.scalar.activation(out=avv[:, :, 0:3, :],
                                         in_=sgv[:, :, 0:3, :],
                                         func=AF.Sigmoid)
                    # tgc tiles hold [tanh_g (d,x) | c (d,x)]; this step's
                    # tanh_g lands in the PREVIOUS tile (next to c_{s-1}) so
                    # one DVE computes (i,f) * (tanh_g_s, c_{s-1}).
                    avg = av[:].rearrange("p (d g x) -> p g d x", d=2, g=4)
                    if s == 0:
                        tgc = tgc_p.tile([8, 1024], dt.float32, tag="t")
                        nc.scalar.activation(
                            out=tgc[:, 0:512].rearrange(
                                "p (q d x) -> p q d x", q=1, d=2),
                            in_=sgv[:, :, 3:4, :].rearrange(
                                "p d q x -> p q d x"), func=AF.Tanh)
                        nc.vector.tensor_tensor(
                            out=tgc[:, 512:1024].rearrange(
                                "p (q d x) -> p q d x", q=1, d=2),
                            in0=avg[:, 0:1, :, :],
                            in1=tgc[:, 0:512].rearrange(
                                "p (q d x) -> p q d x", q=1, d=2),
                            op=OP.mult)
                    else:
                        prev = tgc_prev[0]
                        nc.scalar.activation(
                            out=prev[:, 0:512].rearrange(
                                "p (q d x) -> p q d x", q=1, d=2),
                            in_=sgv[:, :, 3:4, :].rearrange(
                                "p d q x -> p q d x"), func=AF.Tanh)
                        tt = tt_p.tile([8, 1024], dt.float32, tag="tt")
                        nc.vector.tensor_tensor(
                            out=tt[:].rearrange("p (G d x) -> p G d x",
                                                G=2, d=2),
                            in0=avg[:, 0:2, :, :],
                            in1=prev[:].rearrange("p (G d x) -> p G d x",
                                                  G=2, d=2),
                            op=OP.mult)
                        tgc = tgc_p.tile([8, 1024], dt.float32, tag="t")
                        nc.vector.tensor_tensor(
                            out=tgc[:, 512:1024], in0=tt[:, 0:512],
                            in1=tt[:, 512:1024], op=OP.add)
                    tgc_prev[0] = tgc
                    tc = tc_p.tile([8, 512], dt.float32, tag="c")
                    nc.scalar.activation(out=tc[:], in_=tgc[:, 512:1024],
                                         func=AF.Tanh)
                    h_sb = h_sb_p.tile([8, 512], dt.bfloat16, tag="h")
                    nc.vector.tensor_tensor(
                        out=h_sb[:].rearrange("p (q d x) -> p q d x",
                                              q=1, d=2),
                        in0=avg[:, 2:3, :, :],
                        in1=tc[:].rearrange("p (q d x) -> p q d x",
                                            q=1, d=2),
                        op=OP.mult)
                    # transpose h -> h_histT
                    tp = tp_p.tile([128, 128], dt.bfloat16, tag="t")
                    for j in range(4):
                        nc.tensor.transpose(
                            out=tp[:, j * 8:(j + 1) * 8],
                            in_=h_sb[:, j * 128:(j + 1) * 128],
                            identity=ident_b[0:8, 0:8])
                    tf = s
                    tb = L_eff - 1 - s
                    nc.scalar.copy(out=h_histT[:, tf * 32:tf * 32 + 16],
                                   in_=tp[:, 0:16])
                    nc.vector.tensor_copy(
                        out=h_histT[:, tb * 32 + 16:tb * 32 + 32],
                        in_=tp[:, 16:32])

                TB = 64 if L_eff >= 64 else L_eff

                def emission():
                    hv = h_histT[:].rearrange("p (t dc b) -> p t dc b",
                                              dc=4, b=8)
                    for blk in range(L_eff // TB):
                        epst = crf_ps_p.tile([128, 512], dt.float32,
                                             tag="c")
                        eps = epst[0:33, :]
                        for kt in range(4):
                            rhs = hv[:, blk * TB:(blk + 1) * TB, kt, :]
                            nc.tensor.matmul(
                                out=eps, lhsT=wlin(kt), rhs=rhs,
                                start=(kt == 0), stop=(kt == 3))
                        nc.scalar.activation(
                            out=emisT[:, blk * TB * 8:(blk + 1) * TB * 8],
                            in_=eps, func=AF.Identity, bias=ebias[:])
                    for i in range(NTOK // 128):
                        lh = emisT[:, i * 128:(i + 1) * 128]
                        csb = crf_sb_p.tile([128, 1024], dt.float32, tag="c")
                        for hf in range(2):
                            cps = crf_ps_p.tile([128, 512], dt.float32,
                                                tag="c")
                            nc.tensor.matmul(out=cps[:], lhsT=lh,
                                             rhs=jsel[:, hf * 512:
                                                      hf * 512 + 512],
                                             start=True, stop=True)
                            if hf == 0:
                                nc.scalar.copy(out=csb[:, 0:512], in_=cps[:])
                            else:
                                nc.vector.tensor_copy(out=csb[:, 512:1024],
                                                      in_=cps[:])
                        t0 = i * 16
                        dst = d_out[:, t0:t0 + 16, :].rearrange(
                            "b t o -> t b o")
                        nc.sync.dma_start(out=dst, in_=csb[:])

                for _rep in range(reps):
                    tgc_prev[0] = None
                    P_flat[0] = P_flat[1] = None
                    for s in range(L_eff):
                        scan_step(s)
                    emission()

                if timing:
                    tl = crf_sb_p.tile([1, 16], dt.float32, tag="tl")
                    nc.sync.dma_start(out=tl[:], in_=d_out[0, 0, 0:16])
                    nc.sync.dma_start(out=d_out_ext[:], in_=tl[:])

    nc.finalize()
    return nc


_CACHE = {}


def _get_nc(L_eff=L, has_bias=True):
    key = (L_eff, has_bias)
    if key not in _CACHE:
        _CACHE[key] = build_nc(L_eff, has_bias=has_bias)
    return _CACHE[key]


def kernel(**inputs):
    from concourse.bass_utils import run_bass_kernel_spmd

    in_maps = _host_prep(inputs, L)
    has_bias = bool(np.any(in_maps[0]["cb"]))
    nc = _get_nc(L, has_bias)
    res = run_bass_kernel_spmd(nc, in_maps, list(range(NCORES)))
    outs = [res.results[c]["out"] for c in range(NCORES)]
    # under axon the per-core results are views of one contiguous download;
    # reuse it instead of re-copying 134MB
    base = outs[0].base
    if (isinstance(base, np.ndarray) and base.dtype == np.float32
            and base.shape == (NCORES, BC, L, 1024)
            and all(outs[c].base is base for c in range(NCORES))
            and base.flags.c_contiguous):
        return base.reshape(B, L, OUT, OUT)
    return np.concatenate(outs, axis=0).reshape(B, L, OUT, OUT)


if __name__ == "__main__":
    nc = build_nc(64)
    print("built OK")


# revision 15
# speedup vs baseline: 1.7280x; 1.7280x over previous
"""BiLSTM-CRF Trainium2 kernel (flip-layout v2).

Full-input contract: kernel(**inputs) takes the unsharded numpy inputs and
returns the full [64, 512, 32, 32] float32 output. Batch (64) is sharded
across 8 NeuronCores (8 sentences per core); the embedding lookup runs on
the host (so only the gathered activations cross the wire, not the 15MB
table replicated 8x), the BiLSTM scan + emission + CRF expand run on
device, and per-core outputs are reassembled zero-copy.

Device pipeline per core (instruction-count-minimal layout):
  - x upload pre-gathered as bf16 [L*8, 128] (t-major); PE-transposed into
    xT [128 = E, L*8].
  - P windows (16 steps = 128 tokens): P[tok, gate] = x @ Wih.T + b via 2
    matmuls per 512-col half, PSUM -> SBUF fp32. Gate col order (i,f,o,g).
    The bwd direction reuses the fwd xT: window w reads fwd window
    NWIN-1-w and steps index its partition rows mirrored.
  - Scan step (both dirs): gates[8b, 1024] = h_prev.T @ Whh (4 matmuls per
    dir: 2 k-chunks x 2 psum-bank halves), + P row-slice via one DVE add
    per dir into gates_sb [8, 2048] = (dir, gate, 256). One sigmoid ACT
    covers i,f,o of both dirs (strided AP); one tanh ACT for g. LSTM cell
    update in 3 DVE ops on (gate, dir, x) views; h [8, 512] bf16 is
    PE-transposed (4x [8,128]) into h_histT [128, (t, dc4, b8)] with the
    bwd half written at slot L-1-s.
  - Emission: per 64-t block, 4 accumulating matmuls (k-chunks = dir x
    h-chunk) -> emisT [33, L*8] (row 32 == 1 via ACT bias trick).
  - CRF expand: per 128-token tile, 2 matmuls lhsT=emisT-slice vs
    Jsel_aug halves (row 32 carries transition + b_lin), 2 copies, 1 DMA.
"""

import numpy as np

VOCAB, EMB, HID, OUT = 30000, 128, 256, 32
B, L = 64, 512
NCORES = 8
BC = B // NCORES  # sentences per core = 8


# ---------------------------------------------------------------------------
# host-side prep

_WCACHE = {}


def _fingerprint(a):
    a = np.ascontiguousarray(a)
    v = a.view(np.uint8)
    return (a.shape, a.dtype.str, bytes(v.reshape(-1)[:: max(1, v.size // 64)][:64]))


def _prep_weights(inputs):
    """Pack weights (bf16) + Jsel/bias constants. Cached on input identity."""
    import ml_dtypes

    key = tuple(inputs[k].__array_interface__["data"][0]
                for k in ("Wih_f", "Whh_f", "W_lin", "transition", "embedding"))
    fp = (_fingerprint(inputs["W_lin"]), _fingerprint(inputs["Whh_f"]))
    ent = _WCACHE.get(key)
    if ent is not None and ent["fp"] == fp:
        return ent

    bf16 = ml_dtypes.bfloat16
    # torch gate order i,f,g,o -> i,f,o,g
    perm = np.concatenate([np.arange(0, 512), np.arange(768, 1024),
                           np.arange(512, 768)])

    def packdir(Wih, Whh, bih, bhh):
        Wih = np.asarray(Wih, np.float32)[perm]          # [1024, 128]
        Whh = np.asarray(Whh, np.float32)[perm]          # [1024, 256]
        b = (np.asarray(bih, np.float32) + np.asarray(bhh, np.float32))[perm]
        return (np.ascontiguousarray(Wih.T),             # [128, 1024]
                np.ascontiguousarray(Whh[:, :128].T),    # [128, 1024]
                np.ascontiguousarray(Whh[:, 128:].T),    # [128, 1024]
                b)

    wf = packdir(inputs["Wih_f"], inputs["Whh_f"], inputs["bih_f"],
                 inputs["bhh_f"])
    wb = packdir(inputs["Wih_b"], inputs["Whh_b"], inputs["bih_b"],
                 inputs["bhh_b"])

    W_lin = np.asarray(inputs["W_lin"], np.float32)       # [32, 512]
    b_lin = np.asarray(inputs["b_lin"], np.float32)
    trans = np.asarray(inputs["transition"], np.float32)

    WlinT = np.ascontiguousarray(W_lin.T)                 # [512, 32]
    Wlin_aug = np.zeros([4, 128, 33], np.float32)
    for kt in range(4):
        Wlin_aug[kt, :, :32] = WlinT[kt * 128:(kt + 1) * 128]

    # cw: [128, 6*1024 + 4*33] bf16
    cw = np.concatenate(
        [wf[0], wb[0], wf[1], wf[2], wb[1], wb[2],
         Wlin_aug.transpose(1, 0, 2).reshape(128, 132)], axis=1)
    cw = np.ascontiguousarray(cw).astype(bf16)

    cb = np.concatenate([wf[3], wb[3]])[None, :]          # [1, 2048]
    cb = np.ascontiguousarray(cb).astype(bf16)

    Jsel_aug = np.zeros([33, 1024], np.float32)
    for j in range(32):
        Jsel_aug[j, np.arange(32) * 32 + j] = 1.0
    Jsel_aug[32] = (trans + b_lin[None, :]).reshape(-1)
    emis_bias = np.zeros([33, 1], np.float32)
    emis_bias[32, 0] = 1.0
    cj = np.ascontiguousarray(np.concatenate([Jsel_aug, emis_bias], axis=1))

    emb_bf = np.asarray(inputs["embedding"], np.float32).astype(bf16)

    ent = {"cw": cw, "cb": cb, "cj": cj, "emb_bf": emb_bf, "fp": fp}
    _WCACHE[key] = ent
    return ent


def _host_prep(inputs, L_eff=L):
    ent = _prep_weights(inputs)
    sents = np.asarray(inputs["sents_tensor"])
    if sents.dtype != np.int64 and sents.dtype != np.int32:
        sents = sents.astype(np.int64)
    emb_bf = ent["emb_bf"]
    in_maps = []
    for c in range(NCORES):
        idx = sents[c * BC:(c + 1) * BC, :L_eff].T.reshape(-1)  # (t, b)
        x = np.ascontiguousarray(emb_bf[idx])          # [L*8, 128] bf16
        in_maps.append({"x": x, "cw": ent["cw"], "cb": ent["cb"],
                        "cj": ent["cj"]})
    return in_maps


# ---------------------------------------------------------------------------
# device program

def build_nc(L_eff=L, reps=1, timing=False, has_bias=True):
    import concourse.bass as bass  # noqa: F401
    import concourse.mybir as mybir
    import concourse.tile as tile
    from concourse.bacc import Bacc
    from concourse.masks import make_identity

    dt = mybir.dt
    AF = mybir.ActivationFunctionType
    OP = mybir.AluOpType

    NTOK = L_eff * BC
    WIN = 8 if L_eff >= 8 else L_eff      # steps per P window
    NWIN = L_eff // WIN
    NTW = WIN * 8                         # tokens per window
    NT = NTOK // 128                      # x tiles

    nc = Bacc()

    d_x = nc.declare_dram_parameter("x", [NTOK, 128], dt.bfloat16, False)
    d_cw = nc.declare_dram_parameter("cw", [128, 6 * 1024 + 132],
                                     dt.bfloat16, False)
    d_cb = nc.declare_dram_parameter("cb", [1, 2048], dt.bfloat16, False)
    d_cj = nc.declare_dram_parameter("cj", [33, 1025], dt.float32r, False)
    if timing:
        d_out = nc.dram_tensor("outt", [BC, L_eff, 1024], dt.float32)
        d_out_ext = nc.declare_dram_parameter("out", [1, 16], dt.float32,
                                              isOutput=True)
    else:
        d_out = nc.declare_dram_parameter("out", [BC, L_eff, 1024],
                                          dt.float32, isOutput=True)
        d_out_ext = None
    d_pb = [nc.dram_tensor(f"pb{d}", [WIN, 8, 1024], dt.bfloat16)
            for d in range(2)]
    d_hb = [nc.dram_tensor(f"hb{d}", [2, 128, 8], dt.bfloat16)
            for d in range(2)]

    with tile.TileContext(nc) as tc:
        with (
            tc.tile_pool(name="const", bufs=1) as const,
            tc.tile_pool(name="state", bufs=1) as state,
        ):
            ident = const.tile([128, 128], dt.float32)
            make_identity(nc, ident[:])
            ident_b = const.tile([128, 128], dt.bfloat16)
            nc.vector.tensor_copy(out=ident_b[:], in_=ident[:])
            ones1 = const.tile([1, 128], dt.bfloat16)
            nc.vector.memset(ones1[:], 1.0)

            cw_sb = const.tile([128, 6 * 1024 + 132], dt.bfloat16)
            nc.sync.dma_start(out=cw_sb[:], in_=d_cw[:])
            cb_sb = const.tile([1, 2048], dt.bfloat16)
            nc.sync.dma_start(out=cb_sb[:], in_=d_cb[:])
            cj_sb = const.tile([33, 1025], dt.float32r)
            nc.sync.dma_start(out=cj_sb[:], in_=d_cj[:])
            jsel = cj_sb[:, 0:1024]
            ebias = cj_sb[:, 1024:1025].bitcast(dt.float32)

            def wih(d):
                return cw_sb[:, d * 1024:(d + 1) * 1024]

            def whh(d, k):
                off = 2048 + (d * 2 + k) * 1024
                return cw_sb[:, off:off + 1024]

            def wlin(kt):
                off = 6144 + kt * 33
                return cw_sb[:, off:off + 33]

            xT = state.tile([128, NTOK], dt.bfloat16)
            h_histT = state.tile([128, L_eff * 32], dt.bfloat16)
            emisT = state.tile([33, NTOK], dt.float32r)

            with (
                tc.tile_pool(name="gat", bufs=1) as gat,
                tc.tile_pool(name="tp", bufs=1, space="PSUM") as tp_p,
                tc.tile_pool(name="pw", bufs=1, space="PSUM") as pw_ps_p,
                tc.tile_pool(name="gps", bufs=1, space="PSUM") as g_ps_p,
                tc.tile_pool(name="cps", bufs=1, space="PSUM") as crf_ps_p,
                tc.tile_pool(name="psb", bufs=1) as p_sb_p,
                tc.tile_pool(name="pfl", bufs=1) as p_fl_p,
                tc.tile_pool(name="sg", bufs=1) as sg_p,
                tc.tile_pool(name="av", bufs=1) as av_p,
                tc.tile_pool(name="tt", bufs=1) as tt_p,
                tc.tile_pool(name="tgc", bufs=2) as tgc_p,
                tc.tile_pool(name="tc2", bufs=1) as tc_p,
                tc.tile_pool(name="hsb", bufs=2) as h_sb_p,
                tc.tile_pool(name="csb", bufs=2) as crf_sb_p,
            ):
                # ---- x load + transpose into xT ----
                if timing:
                    nc.vector.memset(xT[:], 0.0)
                ngrp = 0 if timing else (NT + 7) // 8
                for g in range(ngrp):
                    a0 = g * 8
                    na = min(8, NT - a0)
                    gt = gat.tile([128, 128 * na], dt.bfloat16, tag="g")
                    src = d_x[:].rearrange("(a p) e -> p a e", p=128)
                    nc.sync.dma_start(
                        out=gt[:].rearrange("p (a e) -> p a e", e=128),
                        in_=src[:, a0:a0 + na, :])
                    for a in range(na):
                        pt = tp_p.tile([128, 128], dt.bfloat16, tag="t")
                        nc.tensor.transpose(
                            out=pt[:], in_=gt[:, a * 128:(a + 1) * 128],
                            identity=ident_b[:])
                        col = (a0 + a) * 128
                        if a % 2 == 0:
                            nc.vector.tensor_copy(
                                out=xT[:, col:col + 128], in_=pt[:])
                        else:
                            nc.scalar.copy(
                                out=xT[:, col:col + 128], in_=pt[:])

                P_flat = [None, None]

                def jit_window(d, w):
                    """P window -> P_flat[d] [8 b, WIN*1024] bf16.

                    Engine reads need 32-aligned partition bases, so the
                    per-step P row-slices of the [128 tok, 1024] PSUM tile
                    are shuffled to batch-partition layout via a DRAM
                    bounce (DMA moves across partitions freely).
                    """
                    win = w if d == 0 else NWIN - 1 - w
                    ps = pw_ps_p.tile([NTW, 1024], dt.float32, tag="P")
                    for hf in range(2):
                        o = hf * 512
                        if has_bias:
                            nc.tensor.matmul(
                                out=ps[:, o:o + 512], lhsT=ones1[:, 0:NTW],
                                rhs=cb_sb[0:1,
                                          d * 1024 + o:d * 1024 + o + 512],
                                start=True, stop=False)
                        nc.tensor.matmul(
                            out=ps[:, o:o + 512],
                            lhsT=xT[:, win * NTW:(win + 1) * NTW],
                            rhs=wih(d)[:, o:o + 512],
                            start=not has_bias, stop=True)
                    sb = p_sb_p.tile([NTW, 1024], dt.bfloat16, tag=f"p{d}")
                    if d == 0:
                        nc.scalar.copy(out=sb[:], in_=ps[:])
                    else:
                        nc.vector.tensor_copy(out=sb[:], in_=ps[:])
                    nc.sync.dma_start(
                        out=d_pb[d][:].rearrange("t b g -> (t b) g"),
                        in_=sb[:])
                    fl = p_fl_p.tile([8, WIN * 1024], dt.bfloat16,
                                     tag=f"f{d}")
                    nc.sync.dma_start(
                        out=fl[:].rearrange("p (t g) -> p t g", g=1024),
                        in_=d_pb[d][:].rearrange("t b g -> b t g"))
                    P_flat[d] = fl

                tgc_prev = [None]

                def scan_step(s):
                    t_l = s % WIN
                    if t_l == 0:
                        jit_window(0, s // WIN)
                        jit_window(1, s // WIN)
                    colf = slice(t_l * 1024, t_l * 1024 + 1024)
                    colb = slice((WIN - 1 - t_l) * 1024,
                                 (WIN - t_l) * 1024)
                    sg = sg_p.tile([8, 2048], dt.float32, tag="s")
                    if s == 0:
                        nc.scalar.copy(out=sg[:, 0:1024],
                                       in_=P_flat[0][:, colf])
                        nc.vector.tensor_copy(out=sg[:, 1024:2048],
                                              in_=P_flat[1][:, colb])
                        gps2 = None
                    else:
                        # both dirs' matmuls first, then both DVE adds:
                        # engine switches cost ~25us each on this backend
                        gps2 = []
                        for d in range(2):
                            slot = (s - 1) if d == 0 else (L_eff - s)
                            g_ps = g_ps_p.tile([8, 1024], dt.float32,
                                               tag=f"g{d}")
                            for hf in range(2):
                                o = hf * 512
                                for k in range(2):
                                    cc = slot * 32 + (d * 2 + k) * 8
                                    nc.tensor.matmul(
                                        out=g_ps[:, o:o + 512],
                                        lhsT=h_histT[:, cc:cc + 8],
                                        rhs=whh(d, k)[:, o:o + 512],
                                        start=(k == 0), stop=(k == 1))
                            gps2.append(g_ps)
                        for d in range(2):
                            col = colf if d == 0 else colb
                            nc.vector.tensor_tensor(
                                out=sg[:, d * 1024:(d + 1) * 1024],
                                in0=gps2[d][:], in1=P_flat[d][:, col],
                                op=OP.add)
                    # nonlinearities; sg layout (d2, g4, x256), g = i,f,o,g
                    av = av_p.tile([8, 2048], dt.float32, tag="a")
                    sgv = sg[:].rearrange("p (d g x) -> p d g x", d=2, g=4)
                    avv = av[:].rearrange("p (d g x) -> p d g x", d=2, g=4)
                    nc.scalar.activation(out=avv[:, :, 0:3, :],
                                         in_=sgv[:, :, 0:3, :],
                                         func=AF.Sigmoid)
                    # tgc tiles hold [tanh_g (d,x) | c (d,x)]; this step's
                    # tanh_g lands in the PREVIOUS tile (next to c_{s-1}) so
                    # one DVE computes (i,f) * (tanh_g_s, c_{s-1}).
                    avg = av[:].rearrange("p (d g x) -> p g d x", d=2, g=4)
                    if s == 0:
                        tgc = tgc_p.tile([8, 1024], dt.float32, tag="t")
                        nc.scalar.activation(
                            out=tgc[:, 0:512].rearrange(
                                "p (q d x) -> p q d x", q=1, d=2),
                            in_=sgv[:, :, 3:4, :].rearrange(
                                "p d q x -> p q d x"), func=AF.Tanh)
                        nc.vector.tensor_tensor(
                            out=tgc[:, 512:1024].rearrange(
                                "p (q d x) -> p q d x", q=1, d=2),
                            in0=avg[:, 0:1, :, :],
                            in1=tgc[:, 0:512].rearrange(
                                "p (q d x) -> p q d x", q=1, d=2),
                            op=OP.mult)
                    else:
                        prev = tgc_prev[0]
                        nc.scalar.activation(
                            out=prev[:, 0:512].rearrange(
                                "p (q d x) -> p q d x", q=1, d=2),
                            in_=sgv[:, :, 3:4, :].rearrange(
                                "p d q x -> p q d x"), func=AF.Tanh)
                        tt = tt_p.tile([8, 1024], dt.float32, tag="tt")
                        nc.vector.tensor_tensor(
                            out=tt[:].rearrange("p (G d x) -> p G d x",
                                                G=2, d=2),
                            in0=avg[:, 0:2, :, :],
                            in1=prev[:].rearrange("p (G d x) -> p G d x",
                                                  G=2, d=2),
                            op=OP.mult)
                        tgc = tgc_p.tile([8, 1024], dt.float32, tag="t")
                        nc.vector.tensor_tensor(
                            out=tgc[:, 512:1024], in0=tt[:, 0:512],
                            in1=tt[:, 512:1024], op=OP.add)
                    tgc_prev[0] = tgc
                    tc = tc_p.tile([8, 512], dt.float32, tag="c")
                    nc.scalar.activation(out=tc[:], in_=tgc[:, 512:1024],
                                         func=AF.Tanh)
                    h_sb = h_sb_p.tile([8, 512], dt.bfloat16, tag="h")
                    nc.vector.tensor_tensor(
                        out=h_sb[:].rearrange("p (q d x) -> p q d x",
                                              q=1, d=2),
                        in0=avg[:, 2:3, :, :],
                        in1=tc[:].rearrange("p (q d x) -> p q d x",
                                            q=1, d=2),
                        op=OP.mult)
                    # transpose h -> h_histT
                    # transpose h into h_histT via DRAM bounce: 4 DMAs
                    # beat 4 PE transposes + 2 copies on this backend
                    tf = s
                    tb = L_eff - 1 - s
                    for d in range(2):
                        nc.sync.dma_start(
                            out=d_hb[d][:].rearrange("k x b -> b k x"),
                            in_=h_sb[:, d * 256:(d + 1) * 256].rearrange(
                                "b (k x) -> b k x", k=2))
                    for d, t in ((0, tf), (1, tb)):
                        c0 = t * 32 + d * 16
                        nc.sync.dma_start(
                            out=h_histT[:, c0:c0 + 16].rearrange(
                                "x (k b) -> x k b", k=2),
                            in_=d_hb[d][:].rearrange("k x b -> x k b"))

                TB = 64 if L_eff >= 64 else L_eff

                def emission():
                    hv = h_histT[:].rearrange("p (t dc b) -> p t dc b",
                                              dc=4, b=8)
                    for blk in range(L_eff // TB):
                        epst = crf_ps_p.tile([128, 512], dt.float32,
                                             tag="c")
                        eps = epst[0:33, :]
                        for kt in range(4):
                            rhs = hv[:, blk * TB:(blk + 1) * TB, kt, :]
                            nc.tensor.matmul(
                                out=eps, lhsT=wlin(kt), rhs=rhs,
                                start=(kt == 0), stop=(kt == 3))
                        nc.scalar.activation(
                            out=emisT[:, blk * TB * 8:(blk + 1) * TB * 8],
                            in_=eps, func=AF.Identity, bias=ebias[:])
                    for i in range(NTOK // 128):
                        lh = emisT[:, i * 128:(i + 1) * 128]
                        csb = crf_sb_p.tile([128, 1024], dt.float32, tag="c")
                        for hf in range(2):
                            cps = crf_ps_p.tile([128, 512], dt.float32,
                                                tag="c")
                            nc.tensor.matmul(out=cps[:], lhsT=lh,
                                             rhs=jsel[:, hf * 512:
                                                      hf * 512 + 512],
                                             start=True, stop=True)
                            if hf == 0:
                                nc.scalar.copy(out=csb[:, 0:512], in_=cps[:])
                            else:
                                nc.vector.tensor_copy(out=csb[:, 512:1024],
                                                      in_=cps[:])
                        t0 = i * 16
                        dst = d_out[:, t0:t0 + 16, :].rearrange(
                            "b t o -> t b o")
                        nc.sync.dma_start(out=dst, in_=csb[:])

                for _rep in range(reps):
                    tgc_prev[0] = None
                    P_flat[0] = P_flat[1] = None
                    for s in range(L_eff):
                        scan_step(s)
                    emission()

                if timing:
                    tl = crf_sb_p.tile([1, 16], dt.float32, tag="tl")
                    nc.sync.dma_start(out=tl[:], in_=d_out[0, 0, 0:16])
                    nc.sync.dma_start(out=d_out_ext[:], in_=tl[:])

    nc.finalize()
    return nc


_CACHE = {}


def _get_nc(L_eff=L, has_bias=True):
    key = (L_eff, has_bias)
    if key not in _CACHE:
        _CACHE[key] = build_nc(L_eff, has_bias=has_bias)
    return _CACHE[key]


def kernel(**inputs):
    from concourse.bass_utils import run_bass_kernel_spmd

    in_maps = _host_prep(inputs, L)
    has_bias = bool(np.any(in_maps[0]["cb"]))
    nc = _get_nc(L, has_bias)
    res = run_bass_kernel_spmd(nc, in_maps, list(range(NCORES)))
    outs = [res.results[c]["out"] for c in range(NCORES)]
    # under axon the per-core results are views of one contiguous download;
    # reuse it instead of re-copying 134MB
    base = outs[0].base
    if (isinstance(base, np.ndarray) and base.dtype == np.float32
            and base.shape == (NCORES, BC, L, 1024)
            and all(outs[c].base is base for c in range(NCORES))
            and base.flags.c_contiguous):
        return base.reshape(B, L, OUT, OUT)
    return np.concatenate(outs, axis=0).reshape(B, L, OUT, OUT)


if __name__ == "__main__":
    nc = build_nc(64)
    print("built OK")


# revision 16
# speedup vs baseline: 1.8974x; 1.0980x over previous
"""BiLSTM-CRF Trainium2 kernel (flip-layout v2).

Full-input contract: kernel(**inputs) takes the unsharded numpy inputs and
returns the full [64, 512, 32, 32] float32 output. Batch (64) is sharded
across 8 NeuronCores (8 sentences per core); the embedding lookup runs on
the host (so only the gathered activations cross the wire, not the 15MB
table replicated 8x), the BiLSTM scan + emission + CRF expand run on
device, and per-core outputs are reassembled zero-copy.

Device pipeline per core (instruction-count-minimal layout):
  - x upload pre-gathered as bf16 [L*8, 128] (t-major); PE-transposed into
    xT [128 = E, L*8].
  - P windows (16 steps = 128 tokens): P[tok, gate] = x @ Wih.T + b via 2
    matmuls per 512-col half, PSUM -> SBUF fp32. Gate col order (i,f,o,g).
    The bwd direction reuses the fwd xT: window w reads fwd window
    NWIN-1-w and steps index its partition rows mirrored.
  - Scan step (both dirs): gates[8b, 1024] = h_prev.T @ Whh (4 matmuls per
    dir: 2 k-chunks x 2 psum-bank halves), + P row-slice via one DVE add
    per dir into gates_sb [8, 2048] = (dir, gate, 256). One sigmoid ACT
    covers i,f,o of both dirs (strided AP); one tanh ACT for g. LSTM cell
    update in 3 DVE ops on (gate, dir, x) views; h [8, 512] bf16 is
    PE-transposed (4x [8,128]) into h_histT [128, (t, dc4, b8)] with the
    bwd half written at slot L-1-s.
  - Emission: per 64-t block, 4 accumulating matmuls (k-chunks = dir x
    h-chunk) -> emisT [33, L*8] (row 32 == 1 via ACT bias trick).
  - CRF expand: per 128-token tile, 2 matmuls lhsT=emisT-slice vs
    Jsel_aug halves (row 32 carries transition + b_lin), 2 copies, 1 DMA.
"""

import numpy as np

VOCAB, EMB, HID, OUT = 30000, 128, 256, 32
B, L = 64, 512
NCORES = 8
BC = B // NCORES  # sentences per core = 8


# ---------------------------------------------------------------------------
# host-side prep

_WCACHE = {}


def _fingerprint(a):
    a = np.ascontiguousarray(a)
    v = a.view(np.uint8)
    return (a.shape, a.dtype.str, bytes(v.reshape(-1)[:: max(1, v.size // 64)][:64]))


def _prep_weights(inputs):
    """Pack weights (bf16) + Jsel/bias constants. Cached on input identity."""
    import ml_dtypes

    key = tuple(inputs[k].__array_interface__["data"][0]
                for k in ("Wih_f", "Whh_f", "W_lin", "transition", "embedding"))
    fp = (_fingerprint(inputs["W_lin"]), _fingerprint(inputs["Whh_f"]))
    ent = _WCACHE.get(key)
    if ent is not None and ent["fp"] == fp:
        return ent

    bf16 = ml_dtypes.bfloat16
    # torch gate order i,f,g,o -> i,f,o,g
    perm = np.concatenate([np.arange(0, 512), np.arange(768, 1024),
                           np.arange(512, 768)])

    def packdir(Wih, Whh, bih, bhh):
        Wih = np.asarray(Wih, np.float32)[perm]          # [1024, 128]
        Whh = np.asarray(Whh, np.float32)[perm]          # [1024, 256]
        b = (np.asarray(bih, np.float32) + np.asarray(bhh, np.float32))[perm]
        return (np.ascontiguousarray(Wih.T),             # [128, 1024]
                np.ascontiguousarray(Whh[:, :128].T),    # [128, 1024]
                np.ascontiguousarray(Whh[:, 128:].T),    # [128, 1024]
                b)

    wf = packdir(inputs["Wih_f"], inputs["Whh_f"], inputs["bih_f"],
                 inputs["bhh_f"])
    wb = packdir(inputs["Wih_b"], inputs["Whh_b"], inputs["bih_b"],
                 inputs["bhh_b"])

    W_lin = np.asarray(inputs["W_lin"], np.float32)       # [32, 512]
    b_lin = np.asarray(inputs["b_lin"], np.float32)
    trans = np.asarray(inputs["transition"], np.float32)

    WlinT = np.ascontiguousarray(W_lin.T)                 # [512, 32]
    Wlin_aug = np.zeros([4, 128, 33], np.float32)
    for kt in range(4):
        Wlin_aug[kt, :, :32] = WlinT[kt * 128:(kt + 1) * 128]

    # cw: [128, 6*1024 + 4*33] bf16
    cw = np.concatenate(
        [wf[0], wb[0], wf[1], wf[2], wb[1], wb[2],
         Wlin_aug.transpose(1, 0, 2).reshape(128, 132)], axis=1)
    cw = np.ascontiguousarray(cw).astype(bf16)

    cb = np.concatenate([wf[3], wb[3]])[None, :]          # [1, 2048]
    cb = np.ascontiguousarray(cb).astype(bf16)

    Jsel_aug = np.zeros([33, 1024], np.float32)
    for j in range(32):
        Jsel_aug[j, np.arange(32) * 32 + j] = 1.0
    Jsel_aug[32] = (trans + b_lin[None, :]).reshape(-1)
    emis_bias = np.zeros([33, 1], np.float32)
    emis_bias[32, 0] = 1.0
    cj = np.ascontiguousarray(np.concatenate([Jsel_aug, emis_bias], axis=1))

    emb_bf = np.asarray(inputs["embedding"], np.float32).astype(bf16)

    ent = {"cw": cw, "cb": cb, "cj": cj, "emb_bf": emb_bf, "fp": fp}
    _WCACHE[key] = ent
    return ent


def _host_prep(inputs, L_eff=L):
    ent = _prep_weights(inputs)
    sents = np.asarray(inputs["sents_tensor"])
    if sents.dtype != np.int64 and sents.dtype != np.int32:
        sents = sents.astype(np.int64)
    emb_bf = ent["emb_bf"]
    in_maps = []
    for c in range(NCORES):
        idx = sents[c * BC:(c + 1) * BC, :L_eff].T.reshape(-1)  # (t, b)
        x = np.ascontiguousarray(emb_bf[idx])          # [L*8, 128] bf16
        in_maps.append({"x": x, "cw": ent["cw"], "cb": ent["cb"],
                        "cj": ent["cj"]})
    return in_maps


# ---------------------------------------------------------------------------
# device program

def build_nc(L_eff=L, reps=1, timing=False, has_bias=True):
    import concourse.bass as bass  # noqa: F401
    import concourse.mybir as mybir
    import concourse.tile as tile
    from concourse.bacc import Bacc
    from concourse.masks import make_identity

    dt = mybir.dt
    AF = mybir.ActivationFunctionType
    OP = mybir.AluOpType

    NTOK = L_eff * BC
    WIN = 8 if L_eff >= 8 else L_eff      # steps per P window
    NWIN = L_eff // WIN
    NTW = WIN * 8                         # tokens per window
    NT = NTOK // 128                      # x tiles

    nc = Bacc()

    d_x = nc.declare_dram_parameter("x", [NTOK, 128], dt.bfloat16, False)
    d_cw = nc.declare_dram_parameter("cw", [128, 6 * 1024 + 132],
                                     dt.bfloat16, False)
    d_cb = nc.declare_dram_parameter("cb", [1, 2048], dt.bfloat16, False)
    d_cj = nc.declare_dram_parameter("cj", [33, 1025], dt.float32r, False)
    if timing:
        d_out = nc.dram_tensor("outt", [BC, L_eff, 1024], dt.float32)
        d_out_ext = nc.declare_dram_parameter("out", [1, 16], dt.float32,
                                              isOutput=True)
    else:
        d_out = nc.declare_dram_parameter("out", [BC, L_eff, 1024],
                                          dt.float32, isOutput=True)
        d_out_ext = None
    d_pb = [nc.dram_tensor(f"pb{d}", [WIN, 8, 1024], dt.bfloat16)
            for d in range(2)]
    d_hb = [nc.dram_tensor(f"hb{d}", [2, 128, 8], dt.bfloat16)
            for d in range(2)]

    with tile.TileContext(nc) as tc:
        with (
            tc.tile_pool(name="const", bufs=1) as const,
            tc.tile_pool(name="state", bufs=1) as state,
        ):
            ident = const.tile([128, 128], dt.float32)
            make_identity(nc, ident[:])
            ident_b = const.tile([128, 128], dt.bfloat16)
            nc.vector.tensor_copy(out=ident_b[:], in_=ident[:])
            ones1 = const.tile([1, 128], dt.bfloat16)
            nc.vector.memset(ones1[:], 1.0)

            cw_sb = const.tile([128, 6 * 1024 + 132], dt.bfloat16)
            nc.sync.dma_start(out=cw_sb[:], in_=d_cw[:])
            cb_sb = const.tile([1, 2048], dt.bfloat16)
            nc.sync.dma_start(out=cb_sb[:], in_=d_cb[:])
            cj_sb = const.tile([33, 1025], dt.float32r)
            nc.sync.dma_start(out=cj_sb[:], in_=d_cj[:])
            jsel = cj_sb[:, 0:1024]
            ebias = cj_sb[:, 1024:1025].bitcast(dt.float32)

            def wih(d):
                return cw_sb[:, d * 1024:(d + 1) * 1024]

            def whh(d, k):
                off = 2048 + (d * 2 + k) * 1024
                return cw_sb[:, off:off + 1024]

            def wlin(kt):
                off = 6144 + kt * 33
                return cw_sb[:, off:off + 33]

            xT = state.tile([128, NTOK], dt.bfloat16)
            h_histT = state.tile([128, L_eff * 32], dt.bfloat16)
            emisT = state.tile([33, NTOK], dt.float32r)

            with (
                tc.tile_pool(name="gat", bufs=1) as gat,
                tc.tile_pool(name="tp", bufs=1, space="PSUM") as tp_p,
                tc.tile_pool(name="pw", bufs=1, space="PSUM") as pw_ps_p,
                tc.tile_pool(name="gps", bufs=1, space="PSUM") as g_ps_p,
                tc.tile_pool(name="cps", bufs=1, space="PSUM") as crf_ps_p,
                tc.tile_pool(name="psb", bufs=2) as p_sb_p,
                tc.tile_pool(name="pfl", bufs=2) as p_fl_p,
                tc.tile_pool(name="sg", bufs=1) as sg_p,
                tc.tile_pool(name="av", bufs=1) as av_p,
                tc.tile_pool(name="tt", bufs=1) as tt_p,
                tc.tile_pool(name="tgc", bufs=2) as tgc_p,
                tc.tile_pool(name="tc2", bufs=1) as tc_p,
                tc.tile_pool(name="hsb", bufs=2) as h_sb_p,
                tc.tile_pool(name="csb", bufs=2) as crf_sb_p,
            ):
                # ---- x load + transpose into xT ----
                if timing:
                    nc.vector.memset(xT[:], 0.0)
                ngrp = 0 if timing else (NT + 7) // 8
                for g in range(ngrp):
                    a0 = g * 8
                    na = min(8, NT - a0)
                    gt = gat.tile([128, 128 * na], dt.bfloat16, tag="g")
                    src = d_x[:].rearrange("(a p) e -> p a e", p=128)
                    nc.sync.dma_start(
                        out=gt[:].rearrange("p (a e) -> p a e", e=128),
                        in_=src[:, a0:a0 + na, :])
                    for a in range(na):
                        pt = tp_p.tile([128, 128], dt.bfloat16, tag="t")
                        nc.tensor.transpose(
                            out=pt[:], in_=gt[:, a * 128:(a + 1) * 128],
                            identity=ident_b[:])
                        col = (a0 + a) * 128
                        if a % 2 == 0:
                            nc.vector.tensor_copy(
                                out=xT[:, col:col + 128], in_=pt[:])
                        else:
                            nc.scalar.copy(
                                out=xT[:, col:col + 128], in_=pt[:])

                P_flat = [None, None]
                P_next = [None, None]

                def jit_window(d, w):
                    """P window -> P_flat[d] [8 b, WIN*1024] bf16.

                    Engine reads need 32-aligned partition bases, so the
                    per-step P row-slices of the [128 tok, 1024] PSUM tile
                    are shuffled to batch-partition layout via a DRAM
                    bounce (DMA moves across partitions freely).
                    """
                    win = w if d == 0 else NWIN - 1 - w
                    ps = pw_ps_p.tile([NTW, 1024], dt.float32, tag="P")
                    for hf in range(2):
                        o = hf * 512
                        if has_bias:
                            nc.tensor.matmul(
                                out=ps[:, o:o + 512], lhsT=ones1[:, 0:NTW],
                                rhs=cb_sb[0:1,
                                          d * 1024 + o:d * 1024 + o + 512],
                                start=True, stop=False)
                        nc.tensor.matmul(
                            out=ps[:, o:o + 512],
                            lhsT=xT[:, win * NTW:(win + 1) * NTW],
                            rhs=wih(d)[:, o:o + 512],
                            start=not has_bias, stop=True)
                    sb = p_sb_p.tile([NTW, 1024], dt.bfloat16, tag=f"p{d}")
                    if d == 0:
                        nc.scalar.copy(out=sb[:], in_=ps[:])
                    else:
                        nc.vector.tensor_copy(out=sb[:], in_=ps[:])
                    nc.sync.dma_start(
                        out=d_pb[d][:].rearrange("t b g -> (t b) g"),
                        in_=sb[:])
                    fl = p_fl_p.tile([8, WIN * 1024], dt.bfloat16,
                                     tag=f"f{d}")
                    nc.sync.dma_start(
                        out=fl[:].rearrange("p (t g) -> p t g", g=1024),
                        in_=d_pb[d][:].rearrange("t b g -> b t g"))
                    P_next[d] = fl

                tgc_prev = [None]

                def scan_step(s):
                    t_l = s % WIN
                    if t_l == 0:
                        if s == 0:
                            jit_window(0, 0)
                            jit_window(1, 0)
                        P_flat[0] = P_next[0]
                        P_flat[1] = P_next[1]
                    # issue the next window mid-window so its PE->ACT->DMA
                    # chain overlaps steps t_l=5..7 instead of stalling the
                    # boundary step
                    if t_l == 4 and s // WIN + 1 < NWIN:
                        jit_window(0, s // WIN + 1)
                        jit_window(1, s // WIN + 1)
                    colf = slice(t_l * 1024, t_l * 1024 + 1024)
                    colb = slice((WIN - 1 - t_l) * 1024,
                                 (WIN - t_l) * 1024)
                    sg = sg_p.tile([8, 2048], dt.float32, tag="s")
                    if s == 0:
                        nc.scalar.copy(out=sg[:, 0:1024],
                                       in_=P_flat[0][:, colf])
                        nc.vector.tensor_copy(out=sg[:, 1024:2048],
                                              in_=P_flat[1][:, colb])
                        gps2 = None
                    else:
                        # both dirs' matmuls first, then both DVE adds:
                        # engine switches cost ~25us each on this backend
                        gps2 = []
                        for d in range(2):
                            slot = (s - 1) if d == 0 else (L_eff - s)
                            g_ps = g_ps_p.tile([8, 1024], dt.float32,
                                               tag=f"g{d}")
                            for hf in range(2):
                                o = hf * 512
                                for k in range(2):
                                    cc = slot * 32 + (d * 2 + k) * 8
                                    nc.tensor.matmul(
                                        out=g_ps[:, o:o + 512],
                                        lhsT=h_histT[:, cc:cc + 8],
                                        rhs=whh(d, k)[:, o:o + 512],
                                        start=(k == 0), stop=(k == 1))
                            gps2.append(g_ps)
                        for d in range(2):
                            col = colf if d == 0 else colb
                            nc.vector.tensor_tensor(
                                out=sg[:, d * 1024:(d + 1) * 1024],
                                in0=gps2[d][:], in1=P_flat[d][:, col],
                                op=OP.add)
                    # nonlinearities; sg layout (d2, g4, x256), g = i,f,o,g
                    av = av_p.tile([8, 2048], dt.float32, tag="a")
                    sgv = sg[:].rearrange("p (d g x) -> p d g x", d=2, g=4)
                    avv = av[:].rearrange("p (d g x) -> p d g x", d=2, g=4)
                    nc.scalar.activation(out=avv[:, :, 0:3, :],
                                         in_=sgv[:, :, 0:3, :],
                                         func=AF.Sigmoid)
                    # tgc tiles hold [tanh_g (d,x) | c (d,x)]; this step's
                    # tanh_g lands in the PREVIOUS tile (next to c_{s-1}) so
                    # one DVE computes (i,f) * (tanh_g_s, c_{s-1}).
                    avg = av[:].rearrange("p (d g x) -> p g d x", d=2, g=4)
                    if s == 0:
                        tgc = tgc_p.tile([8, 1024], dt.float32, tag="t")
                        nc.scalar.activation(
                            out=tgc[:, 0:512].rearrange(
                                "p (q d x) -> p q d x", q=1, d=2),
                            in_=sgv[:, :, 3:4, :].rearrange(
                                "p d q x -> p q d x"), func=AF.Tanh)
                        nc.vector.tensor_tensor(
                            out=tgc[:, 512:1024].rearrange(
                                "p (q d x) -> p q d x", q=1, d=2),
                            in0=avg[:, 0:1, :, :],
                            in1=tgc[:, 0:512].rearrange(
                                "p (q d x) -> p q d x", q=1, d=2),
                            op=OP.mult)
                    else:
                        prev = tgc_prev[0]
                        nc.scalar.activation(
                            out=prev[:, 0:512].rearrange(
                                "p (q d x) -> p q d x", q=1, d=2),
                            in_=sgv[:, :, 3:4, :].rearrange(
                                "p d q x -> p q d x"), func=AF.Tanh)
                        tt = tt_p.tile([8, 1024], dt.float32, tag="tt")
                        nc.vector.tensor_tensor(
                            out=tt[:].rearrange("p (G d x) -> p G d x",
                                                G=2, d=2),
                            in0=avg[:, 0:2, :, :],
                            in1=prev[:].rearrange("p (G d x) -> p G d x",
                                                  G=2, d=2),
                            op=OP.mult)
                        tgc = tgc_p.tile([8, 1024], dt.float32, tag="t")
                        nc.vector.tensor_tensor(
                            out=tgc[:, 512:1024], in0=tt[:, 0:512],
                            in1=tt[:, 512:1024], op=OP.add)
                    tgc_prev[0] = tgc
                    tc = tc_p.tile([8, 512], dt.float32, tag="c")
                    nc.scalar.activation(out=tc[:], in_=tgc[:, 512:1024],
                                         func=AF.Tanh)
                    h_sb = h_sb_p.tile([8, 512], dt.bfloat16, tag="h")
                    nc.vector.tensor_tensor(
                        out=h_sb[:].rearrange("p (q d x) -> p q d x",
                                              q=1, d=2),
                        in0=avg[:, 2:3, :, :],
                        in1=tc[:].rearrange("p (q d x) -> p q d x",
                                            q=1, d=2),
                        op=OP.mult)
                    # transpose h -> h_histT
                    # transpose h into h_histT via DRAM bounce: 4 DMAs
                    # beat 4 PE transposes + 2 copies on this backend
                    tf = s
                    tb = L_eff - 1 - s
                    for d in range(2):
                        nc.sync.dma_start(
                            out=d_hb[d][:].rearrange("k x b -> b k x"),
                            in_=h_sb[:, d * 256:(d + 1) * 256].rearrange(
                                "b (k x) -> b k x", k=2))
                    for d, t in ((0, tf), (1, tb)):
                        c0 = t * 32 + d * 16
                        nc.sync.dma_start(
                            out=h_histT[:, c0:c0 + 16].rearrange(
                                "x (k b) -> x k b", k=2),
                            in_=d_hb[d][:].rearrange("k x b -> x k b"))

                TB = 64 if L_eff >= 64 else L_eff

                def emission():
                    hv = h_histT[:].rearrange("p (t dc b) -> p t dc b",
                                              dc=4, b=8)
                    for blk in range(L_eff // TB):
                        epst = crf_ps_p.tile([128, 512], dt.float32,
                                             tag="c")
                        eps = epst[0:33, :]
                        for kt in range(4):
                            rhs = hv[:, blk * TB:(blk + 1) * TB, kt, :]
                            nc.tensor.matmul(
                                out=eps, lhsT=wlin(kt), rhs=rhs,
                                start=(kt == 0), stop=(kt == 3))
                        nc.scalar.activation(
                            out=emisT[:, blk * TB * 8:(blk + 1) * TB * 8],
                            in_=eps, func=AF.Identity, bias=ebias[:])
                    for i in range(NTOK // 128):
                        lh = emisT[:, i * 128:(i + 1) * 128]
                        csb = crf_sb_p.tile([128, 1024], dt.float32, tag="c")
                        for hf in range(2):
                            cps = crf_ps_p.tile([128, 512], dt.float32,
                                                tag="c")
                            nc.tensor.matmul(out=cps[:], lhsT=lh,
                                             rhs=jsel[:, hf * 512:
                                                      hf * 512 + 512],
                                             start=True, stop=True)
                            if hf == 0:
                                nc.scalar.copy(out=csb[:, 0:512], in_=cps[:])
                            else:
                                nc.vector.tensor_copy(out=csb[:, 512:1024],
                                                      in_=cps[:])
                        t0 = i * 16
                        dst = d_out[:, t0:t0 + 16, :].rearrange(
                            "b t o -> t b o")
                        nc.sync.dma_start(out=dst, in_=csb[:])

                for _rep in range(reps):
                    tgc_prev[0] = None
                    P_flat[0] = P_flat[1] = None
                    P_next[0] = P_next[1] = None
                    for s in range(L_eff):
                        scan_step(s)
                    emission()

                if timing:
                    tl = crf_sb_p.tile([1, 16], dt.float32, tag="tl")
                    nc.sync.dma_start(out=tl[:], in_=d_out[0, 0, 0:16])
                    nc.sync.dma_start(out=d_out_ext[:], in_=tl[:])

    nc.finalize()
    return nc


_CACHE = {}


def _get_nc(L_eff=L, has_bias=True):
    key = (L_eff, has_bias)
    if key not in _CACHE:
        _CACHE[key] = build_nc(L_eff, has_bias=has_bias)
    return _CACHE[key]


def kernel(**inputs):
    from concourse.bass_utils import run_bass_kernel_spmd

    in_maps = _host_prep(inputs, L)
    has_bias = bool(np.any(in_maps[0]["cb"]))
    nc = _get_nc(L, has_bias)
    res = run_bass_kernel_spmd(nc, in_maps, list(range(NCORES)))
    outs = [res.results[c]["out"] for c in range(NCORES)]
    # under axon the per-core results are views of one contiguous download;
    # reuse it instead of re-copying 134MB
    base = outs[0].base
    if (isinstance(base, np.ndarray) and base.dtype == np.float32
            and base.shape == (NCORES, BC, L, 1024)
            and all(outs[c].base is base for c in range(NCORES))
            and base.flags.c_contiguous):
        return base.reshape(B, L, OUT, OUT)
    return np.concatenate(outs, axis=0).reshape(B, L, OUT, OUT)


if __name__ == "__main__":
    nc = build_nc(64)
    print("built OK")
